# revision 43
# baseline (speedup 1.0000x reference)
"""Trainium2 Bass kernel for nn_Net_5488968204310 (gnn_message_passing).

Single-head self-attention (D=128) over N=1024 nodes + gated residual update,
batch B=32, data-parallel across 8 NeuronCores (4 samples per core).

v2 design notes (vs v1 baseline at ~101us):
  - x is uploaded ONCE, bf16, pre-transposed on the host to [B, D, N]; the
    residual add runs in T layout and the output is stored [BPC, D, N] f32,
    un-transposed on the host. No DMA transposes, no f32 x upload: input DMA
    drops 3.15MB -> 1.05MB/core and the 21us DMA head mostly disappears.
  - All 8 128x128 weights packed into ONE [D, 8, D] bf16 tensor = one DMA
    descriptor instead of eight serialized ~600ns descriptor-gens.
  - softmax reciprocal on the DVE (reciprocal_approx_fast, ~18 bits) instead
    of the scalar-engine exp(-ln(den)) trick: -2.5us/sample of ACT time and a
    shorter phase2 chain. Also fixes a read-before-write on the old rb tile.
  - sigmoid(z) = 0.5 + 0.5*tanh(z/2): tanh IS in the exp_and_others ACT table
    (exp is too), so the gate is ONE activation per half instead of the
    3-serial-ACT exp/ln/exp chain; the 0.5 scales fold into Wo/Wo1m/bo_u and
    the +1 folds into the DVE scalar_tensor_tensor that applies the gate.
  - gate/residual tail is fully per-half: two independent chains, each ending
    in its own straight [128,512] f32 store -> short pipeline drain.
  - QK^T stays bf16; AV + denominator stay fp8 DoubleRow (2 rows/col).

v3:
  - u = Wo@attnT + Wo1m@xT and g = Wog2@attnT + Wg1@xT each collapse into ONE
    fp8 DoubleRow matmul: stationary = [W_a; W_x] packed [128,2,128] f8,
    moving = [attnT; x8] packed [128,2,512] f8 - DR's 2-row accumulation IS
    the sum of the two products. Halves the phase3 matmul columns.
  - macro-step emission order P2,P3,P1 so phase2's DVE ops (reciprocal,
    attnT normalize) are not queued behind phase3's six DVE ops - removes
    ~1.2us/sample PE stalls on the pd psum pool.
  - xT DMA split per half so the first q matmul starts ~1us earlier.
"""

import math

import numpy as np
import ml_dtypes

B, N, D = 32, 1024, 128
NCORES = 8
BPC = B // NCORES  # samples per core
NT = N // 128      # node chunks per sample
NW = 8             # packed weights

_CACHE = {}

WNAMES = ["Wq", "Wk", "Wv", "Wg3"]          # bf16 packed, stationary singles
W8NAMES = ["Woh", "Wo1mh", "Wog2", "Wg1"]   # f8 packed, DR pairs (u: 0,1  g: 2,3)


def _bias_mode(vec):
    """(kind, value) where kind in {'zero', 'uniform', 'ap'}."""
    v = np.asarray(vec, np.float32)
    if not np.any(v):
        return ("zero", 0.0)
    if np.all(v == v.flat[0]):
        return ("uniform", float(v.flat[0]))
    return ("ap", 0.0)


def _build_nc(modes):
    import concourse.bacc as bacc
    import concourse.tile as tile
    from concourse import mybir
    from contextlib import ExitStack

    f32 = mybir.dt.float32
    bf16 = mybir.dt.bfloat16
    f8 = mybir.dt.float8e4
    AF = mybir.ActivationFunctionType
    OP = mybir.AluOpType
    DR = mybir.MatmulPerfMode.DoubleRow

    nc = bacc.Bacc("TRN2", target_bir_lowering=False, debug=False)

    xT_d = nc.dram_tensor("xT", [BPC, D, N], bf16, kind="ExternalInput")
    x8_d = nc.dram_tensor("x8", [BPC, D, N], f8, kind="ExternalInput")
    wpk_d = nc.dram_tensor("wpk", [D, 4, D], bf16, kind="ExternalInput")
    wpk8_d = nc.dram_tensor("wpk8", [D, 4, D], f8, kind="ExternalInput")
    out_d = nc.dram_tensor("out", [BPC, D, N], f32, kind="ExternalOutput")
    b_d = {
        n: nc.dram_tensor(n, [D, 1], f32, kind="ExternalInput")
        for n in modes if modes[n][0] == "ap"
    }

    s = 1.0 / math.sqrt(D)

    with tile.TileContext(nc) as tc, ExitStack() as ctx:
        consts = ctx.enter_context(tc.tile_pool(name="consts", bufs=1))
        sb = ctx.enter_context(tc.tile_pool(name="sb", bufs=2))
        sb4 = ctx.enter_context(tc.tile_pool(name="sb4", bufs=4))
        expp = ctx.enter_context(tc.tile_pool(name="expp", bufs=2))
        pw = ctx.enter_context(tc.tile_pool(name="pw", bufs=2, space="PSUM"))
        pd = ctx.enter_context(tc.tile_pool(name="pd", bufs=1, space="PSUM"))
        ph = ctx.enter_context(tc.tile_pool(name="ph", bufs=2, space="PSUM"))

        wpk = consts.tile([D, 4, D], bf16, tag="wpk")
        nc.sync.dma_start(wpk, wpk_d[:, :, :])
        W = {n: wpk[:, i, :] for i, n in enumerate(WNAMES)}
        wpk8 = consts.tile([D, 4, D], f8, tag="wpk8")
        Wu8 = wpk8[:, 0:2, :]   # [Woh; Wo1mh] DR pair
        Wg8 = wpk8[:, 2:4, :]   # [Wog2; Wg1] DR pair
        ones_dr = consts.tile([128, 2, 128], f8, tag="ones_dr")
        nc.vector.memset(ones_dr, 1.0)
        expbias = consts.tile([128, 1], f32, tag="expbias")
        nc.vector.memset(expbias, -2.0)
        BV = {}
        for n in b_d:
            t = consts.tile([D, 1], f32, tag=f"b_{n}")
            nc.sync.dma_start(t, b_d[n][:, :])
            BV[n] = t
        for n, (kind, val) in modes.items():
            if kind == "uniform":
                t = consts.tile([D, 1], f32, tag=f"b_{n}")
                nc.vector.memset(t, val)
                BV[n] = t

        def copyback(dst, src, bname):
            """psum->sbuf copy honoring the bias mode for `bname`."""
            if modes[bname][0] == "zero":
                nc.vector.tensor_copy(dst, src)
            else:
                nc.scalar.activation(dst, src, AF.Identity, bias=BV[bname])

        def act_bias(bname):
            return 0.0 if modes[bname][0] == "zero" else BV[bname]

        ST = {}

        def phase0(b):
            """issue sample b's input DMAs one macro-step ahead."""
            st = {}
            xT = sb4.tile([128, N], bf16, tag="xT")  # [d, n]
            nc.sync.dma_start(xT[:, 0:512], xT_d[b][:, 0:512])
            nc.sync.dma_start(xT[:, 512:1024], xT_d[b][:, 512:1024])
            st["xT"] = xT
            # axT[:, 0, :] <- attnT (phase2); [:, 1, :] <- x in f8: the DR
            # moving pair for the fused u/gate matmuls.
            axT = sb4.tile([128, 2, N], f8, tag="axT")
            nc.sync.dma_start(axT[:, 1, :], x8_d[b])
            st["axT"] = axT
            return st

        def phase1a(st):
            """q/k/v projections + psum evacuation (casts at DVE queue front)."""
            xT = st["xT"]
            p_q = pw.tile([128, N], f32, tag="pw")
            nc.tensor.matmul(p_q[:, 0:512], W["Wq"], xT[:, 0:512], start=True, stop=True)
            nc.tensor.matmul(p_q[:, 512:1024], W["Wq"], xT[:, 512:1024], start=True, stop=True)
            qT = sb.tile([128, N], bf16, tag="qT")
            copyback(qT, p_q, "bq")

            p_k = pw.tile([128, N], f32, tag="pw")
            nc.tensor.matmul(p_k[:, 0:512], W["Wk"], xT[:, 0:512], start=True, stop=True)
            nc.tensor.matmul(p_k[:, 512:1024], W["Wk"], xT[:, 512:1024], start=True, stop=True)
            kT = sb.tile([128, N], bf16, tag="kT")
            copyback(kT, p_k, "bk")

            p_v = pw.tile([128, N], f32, tag="pw")
            for c in range(NT):
                nc.tensor.matmul(p_v[:, c * 128:(c + 1) * 128], xT[:, c * 128:(c + 1) * 128], W["Wv"], start=True, stop=True)
            v_nat = sb.tile([128, NT, 128], f8, tag="v_nat")
            nc.vector.tensor_copy(v_nat, p_v.rearrange("p (c n) -> p c n", c=NT))
            st["qT"], st["kT"], st["v_nat"] = qT, kT, v_nat

        def phase12(stA, stB):
            """Fused: QK^T + exp for sample A, den/recip/AV/normalize for the
            one-step-older sample B, chunk-interleaved on the PE so the DR
            blocks fill the slots where QK would otherwise wait for the
            trailing exp chain."""
            blocks = []
            if stA is not None:
                qT, kT = stA["qT"], stA["kT"]
                # exp output in fp8e4m3: bias -2 rescales exp into fp8 range;
                # the uniform e^-2 cancels between numerator and denominator.
                expw = expp.tile([128, NT, N], f8, tag="expw")  # [m', c_m, q]
                stA["expw"] = expw

                def qk_chunk(c):
                    p_l = pw.tile([128, N], f32, tag="pw")
                    kTc = kT[:, c * 128:(c + 1) * 128]
                    nc.tensor.matmul(p_l[:, 0:512], kTc, qT[:, 0:512], start=True, stop=True)
                    nc.tensor.matmul(p_l[:, 512:1024], kTc, qT[:, 512:1024], start=True, stop=True)
                    nc.scalar.activation(expw[:, c, :], p_l, AF.Exp, scale=s, bias=expbias)

            if stB is not None:
                expwB, v_nat, axT = stB["expw"], stB["v_nat"], stB["axT"]
                rb = sb.tile([128, N], f32, tag="rb")

                def den_block(h):
                    sl = slice(h * 512, (h + 1) * 512)
                    p_dn = pd.tile([128, 512], f32, tag="pden")
                    for c in range(NT // 2):
                        nc.tensor.matmul(
                            p_dn, ones_dr, expwB[:, 2 * c:2 * c + 2, sl],
                            start=(c == 0), stop=(c == NT // 2 - 1),
                            perf_mode=DR,
                        )
                    nc.vector.reciprocal_approx_fast(out=rb[:, sl], in_=p_dn)

                def av_block(h):
                    sl = slice(h * 512, (h + 1) * 512)
                    p_av = pd.tile([128, 512], f32, tag="pav")
                    for c in range(NT // 2):
                        nc.tensor.matmul(
                            p_av, v_nat[:, 2 * c:2 * c + 2, :], expwB[:, 2 * c:2 * c + 2, sl],
                            start=(c == 0), stop=(c == NT // 2 - 1),
                            perf_mode=DR,
                        )
                    nc.vector.tensor_mul(axT[:, 0, sl], p_av, rb[:, sl])

            if stA is not None and stB is not None:
                qk_chunk(0); qk_chunk(1)
                den_block(0)
                qk_chunk(2); qk_chunk(3)
                av_block(0)
                qk_chunk(4); qk_chunk(5)
                den_block(1)
                qk_chunk(6); qk_chunk(7)
                av_block(1)
            elif stA is not None:
                for c in range(NT):
                    qk_chunk(c)
            elif stB is not None:
                den_block(0); av_block(0); den_block(1); av_block(1)

        def phase3a(b, st):
            """gated tail pass A: u/gate matmuls + psum evacuation. Emitted
            FIRST in the macro step so ucopy/relu sit at the DVE queue front;
            the two g3 matmuls borrow the den/av psum banks (free until the
            fused phase12 block later in the step). (GPSIMD cannot access
            PSUM - BIR verifier - so u copy and relu stay on the DVE.)"""
            axT = st["axT"]
            u = sb.tile([128, N], f32, tag="u")
            gp = sb.tile([128, N], bf16, tag="gp")
            t = sb.tile([128, N], bf16, tag="t")
            st["u"], st["t"] = u, t
            for h in range(2):
                sl = slice(h * 512, (h + 1) * 512)
                # u = 0.5*(ret - x): one DR matmul sums Woh@attnT + Wo1mh@x8
                # (Wo, Wo1m, bo_u pre-scaled by 0.5 on the host).
                p_m = ph.tile([128, 512], f32, tag="ph")
                nc.tensor.matmul(p_m, Wu8, axT[:, :, sl], start=True, stop=True, perf_mode=DR)
                copyback(u[:, sl], p_m, "bo_u")

                p_g = ph.tile([128, 512], f32, tag="ph")
                nc.tensor.matmul(p_g, Wg8, axT[:, :, sl], start=True, stop=True, perf_mode=DR)
                if modes["bo_g"][0] == "zero":
                    nc.vector.tensor_scalar(gp[:, sl], p_g, 0.0, None, op0=OP.max)
                else:
                    nc.scalar.activation(gp[:, sl], p_g, AF.Relu, bias=act_bias("bo_g"))
            for h in range(2):
                sl = slice(h * 512, (h + 1) * 512)
                p_g3 = pd.tile([128, 512], f32, tag="pden" if h == 0 else "pav")
                nc.tensor.matmul(p_g3, W["Wg3"], gp[:, sl], start=True, stop=True)
                # gate = sigmoid(z + bg3) = 0.5 + 0.5*tanh((z + bg3)/2);
                # out = x + gate*(ret - x) = x + (1 + tanh(...))*u
                nc.scalar.activation(t[:, sl], p_g3, AF.Tanh, scale=0.5, bias=act_bias("bg3h"))

        def phase3b(b, st):
            """gated tail pass B: apply gate, residual add, store - single
            full-width ops (fewer DVE/DMA instruction overheads; both tanh
            halves are ready by the time the scalar_tensor_tensor runs)."""
            xT, u, t = st["xT"], st["u"], st["t"]
            gd = sb.tile([128, N], f32, tag="gd")
            o = sb.tile([128, N], f32, tag="o")
            nc.vector.scalar_tensor_tensor(
                gd, t, 1.0, u, op0=OP.add, op1=OP.mult,
            )
            nc.gpsimd.tensor_add(o, gd, xT)
            nc.sync.dma_start(out_d[b], o)

        # Software pipeline, depth 4. Per macro step:
        #   P0(k):  input DMAs for sample k (a full step of prefetch),
        #   P3a(k-3): tail matmuls + evacuations (DVE queue front),
        #   P3b(k-3): gate apply + residual + store (gpsimd),
        #   P12: QK^T+exp for k-1 interleaved with den/AV for k-2 - FIRST
        #        among the heavy PE blocks so the 8-exp chain starts early
        #        (it is the longest per-step chain),
        #   P1a(k): q/k/v matmuls + casts at the step's END - their results
        #        are only needed by the next step's P12.
        for k in range(BPC + 3):
            if k < BPC:
                ST[k] = phase0(k)
            if k == 0:
                # deferred so sample 0's xT DMA beats it onto the queue;
                # wpk8 is first needed by phase3a three steps later
                nc.sync.dma_start(wpk8, wpk8_d[:, :, :])
            if 0 <= k - 3:
                phase3a(k - 3, ST[k - 3])
                phase3b(k - 3, ST[k - 3])
            phase12(
                ST[k - 1] if 0 <= k - 1 < BPC else None,
                ST[k - 2] if 0 <= k - 2 < BPC else None,
            )
            if k < BPC:
                phase1a(ST[k])

    # Force Exp and Tanh to resolve to the one table set that holds both
    # (exp_and_others): contents-only lie to the set chooser, dict order
    # (= act_func_set_id) preserved; the set actually loaded at runtime does
    # contain every ACT function used (exp, tanh, relu, identity).
    import concourse.bacc as bacc_mod

    real_get = bacc_mod.get_activation_tables
    target = "exp_and_others"

    def patched_get(arch):
        tabs = real_get(arch)
        strip = {AF.Exp, AF.Tanh}
        return {
            name: (set(fns) if name == target else set(fns) - strip)
            for name, fns in tabs.items()
        }

    bacc_mod.get_activation_tables = patched_get
    try:
        nc.compile()
    finally:
        bacc_mod.get_activation_tables = real_get
    return nc


def _prep_host(inputs):
    """Host-side folds; returns (xT bf16 [B,D,N], packed weights, biases)."""
    f32 = np.float32
    g = {k: np.asarray(v, f32) for k, v in inputs.items()}

    Woh = 0.5 * g["Wo"]
    Wo1mh = 0.5 * (g["Wo1"] - np.eye(D, dtype=f32))
    Wog2 = g["Wo"] @ g["Wg2"]                      # msg path folded into gate
    bo_msg = g["bo"] + g["bv"] @ g["Wo"]           # bv folded through Wo
    bo_u = 0.5 * (bo_msg + g["bo1"])               # 0.5*(msg bias + ret bias)
    bo_g = bo_msg @ g["Wg2"] + g["bg1"] + g["bg2"]
    bg3h = 0.5 * g["bg3"]

    wmap = {
        "Wq": g["Wq"], "Wk": g["Wk"], "Wv": g["Wv"], "Woh": Woh,
        "Wo1mh": Wo1mh, "Wg1": g["Wg1"], "Wog2": Wog2, "Wg3": g["Wg3"],
    }
    bmap = {
        "bq": g["bq"], "bk": g["bk"],
        "bo_u": bo_u, "bo_g": bo_g, "bg3h": bg3h,
    }
    bf16 = ml_dtypes.bfloat16
    f8 = ml_dtypes.float8_e4m3
    wpk = np.ascontiguousarray(
        np.stack([wmap[n] for n in WNAMES], axis=1).astype(bf16)
    )  # [D, 4, D]
    wpk8 = np.ascontiguousarray(
        np.stack([wmap[n] for n in W8NAMES], axis=1).astype(f8)
    )  # [D, 4, D]
    xt = g["x"].transpose(0, 2, 1)
    xT = np.ascontiguousarray(xt.astype(bf16))  # [B, D, N]
    x8 = np.ascontiguousarray(xt.astype(f8))    # [B, D, N]
    return xT, x8, wpk, wpk8, bmap


def _prep_inputs(inputs):
    xT, x8, wpk, wpk8, bmap = _prep_host(inputs)
    modes = {n: _bias_mode(v) for n, v in bmap.items()}
    base = {"wpk": wpk, "wpk8": wpk8}
    for n, v in bmap.items():
        if modes[n][0] == "ap":
            base[n] = np.ascontiguousarray(v.reshape(D, 1).astype(np.float32))
    in_maps = []
    for c in range(NCORES):
        m = dict(base)
        m["xT"] = np.ascontiguousarray(xT[c * BPC:(c + 1) * BPC])
        m["x8"] = np.ascontiguousarray(x8[c * BPC:(c + 1) * BPC])
        in_maps.append(m)
    return in_maps, modes


def kernel(**inputs):
    from concourse.bass_utils import run_bass_kernel_spmd

    in_maps, modes = _prep_inputs(inputs)
    key = tuple(sorted((n, k[0], k[1]) for n, k in modes.items()))
    if _CACHE.get("key") != key:
        _CACHE["nc"] = _build_nc(modes)
        _CACHE["key"] = key
    nc = _CACHE["nc"]

    res = run_bass_kernel_spmd(nc, in_maps, list(range(NCORES)))
    out = np.concatenate([r["out"] for r in res.results], axis=0)  # [B, D, N]
    return np.ascontiguousarray(out.transpose(0, 2, 1)).astype(np.float32)


# revision 45
# speedup vs baseline: 1.2010x; 1.2010x over previous
"""Trainium2 Bass kernel for nn_Net_5488968204310 (gnn_message_passing).

Single-head self-attention (D=128) over N=1024 nodes + gated residual update,
batch B=32, data-parallel across 8 NeuronCores (4 samples per core).

v2 design notes (vs v1 baseline at ~101us):
  - x is uploaded ONCE, bf16, pre-transposed on the host to [B, D, N]; the
    residual add runs in T layout and the output is stored [BPC, D, N] f32,
    un-transposed on the host. No DMA transposes, no f32 x upload: input DMA
    drops 3.15MB -> 1.05MB/core and the 21us DMA head mostly disappears.
  - All 8 128x128 weights packed into ONE [D, 8, D] bf16 tensor = one DMA
    descriptor instead of eight serialized ~600ns descriptor-gens.
  - softmax reciprocal on the DVE (reciprocal_approx_fast, ~18 bits) instead
    of the scalar-engine exp(-ln(den)) trick: -2.5us/sample of ACT time and a
    shorter phase2 chain. Also fixes a read-before-write on the old rb tile.
  - sigmoid(z) = 0.5 + 0.5*tanh(z/2): tanh IS in the exp_and_others ACT table
    (exp is too), so the gate is ONE activation per half instead of the
    3-serial-ACT exp/ln/exp chain; the 0.5 scales fold into Wo/Wo1m/bo_u and
    the +1 folds into the DVE scalar_tensor_tensor that applies the gate.
  - gate/residual tail is fully per-half: two independent chains, each ending
    in its own straight [128,512] f32 store -> short pipeline drain.
  - QK^T stays bf16; AV + denominator stay fp8 DoubleRow (2 rows/col).

v3:
  - u = Wo@attnT + Wo1m@xT and g = Wog2@attnT + Wg1@xT each collapse into ONE
    fp8 DoubleRow matmul: stationary = [W_a; W_x] packed [128,2,128] f8,
    moving = [attnT; x8] packed [128,2,512] f8 - DR's 2-row accumulation IS
    the sum of the two products. Halves the phase3 matmul columns.
  - macro-step emission order P2,P3,P1 so phase2's DVE ops (reciprocal,
    attnT normalize) are not queued behind phase3's six DVE ops - removes
    ~1.2us/sample PE stalls on the pd psum pool.
  - xT DMA split per half so the first q matmul starts ~1us earlier.
"""

import math

import numpy as np
import ml_dtypes

B, N, D = 32, 1024, 128
NCORES = 8
BPC = B // NCORES  # samples per core
NT = N // 128      # node chunks per sample
NW = 8             # packed weights

_CACHE = {}

WNAMES = ["Wq", "Wk", "Wv", "Wg3"]          # bf16 packed, stationary singles
W8NAMES = ["Woh", "Wo1mh", "Wog2", "Wg1"]   # f8 packed, DR pairs (u: 0,1  g: 2,3)


def _bias_mode(vec):
    """(kind, value) where kind in {'zero', 'uniform', 'ap'}."""
    v = np.asarray(vec, np.float32)
    if not np.any(v):
        return ("zero", 0.0)
    if np.all(v == v.flat[0]):
        return ("uniform", float(v.flat[0]))
    return ("ap", 0.0)


def _build_nc(modes):
    import concourse.bacc as bacc
    import concourse.tile as tile
    from concourse import mybir
    from contextlib import ExitStack

    f32 = mybir.dt.float32
    bf16 = mybir.dt.bfloat16
    f8 = mybir.dt.float8e4
    AF = mybir.ActivationFunctionType
    OP = mybir.AluOpType
    DR = mybir.MatmulPerfMode.DoubleRow

    nc = bacc.Bacc("TRN2", target_bir_lowering=False, debug=False)

    xT_d = nc.dram_tensor("xT", [BPC, D, N], bf16, kind="ExternalInput")
    x8_d = nc.dram_tensor("x8", [BPC, D, N], f8, kind="ExternalInput")
    wpk_d = nc.dram_tensor("wpk", [D, 4, D], bf16, kind="ExternalInput")
    wpk8_d = nc.dram_tensor("wpk8", [D, 4, D], f8, kind="ExternalInput")
    out_d = nc.dram_tensor("out", [BPC, D, N], f32, kind="ExternalOutput")
    b_d = {
        n: nc.dram_tensor(n, [D, 1], f32, kind="ExternalInput")
        for n in modes if modes[n][0] == "ap"
    }

    s = 1.0 / math.sqrt(D)

    with tile.TileContext(nc) as tc, ExitStack() as ctx:
        consts = ctx.enter_context(tc.tile_pool(name="consts", bufs=1))
        sb = ctx.enter_context(tc.tile_pool(name="sb", bufs=3))
        sb4 = ctx.enter_context(tc.tile_pool(name="sb4", bufs=4))
        expp = ctx.enter_context(tc.tile_pool(name="expp", bufs=3))
        pw = ctx.enter_context(tc.tile_pool(name="pw", bufs=2, space="PSUM"))
        pd = ctx.enter_context(tc.tile_pool(name="pd", bufs=1, space="PSUM"))
        ph = ctx.enter_context(tc.tile_pool(name="ph", bufs=2, space="PSUM"))

        wpk = consts.tile([D, 4, D], bf16, tag="wpk")
        nc.sync.dma_start(wpk, wpk_d[:, :, :])
        W = {n: wpk[:, i, :] for i, n in enumerate(WNAMES)}
        wpk8 = consts.tile([D, 4, D], f8, tag="wpk8")
        Wu8 = wpk8[:, 0:2, :]   # [Woh; Wo1mh] DR pair
        Wg8 = wpk8[:, 2:4, :]   # [Wog2; Wg1] DR pair
        ones_dr = consts.tile([128, 2, 128], f8, tag="ones_dr")
        nc.vector.memset(ones_dr, 1.0)
        expbias = consts.tile([128, 1], f32, tag="expbias")
        nc.vector.memset(expbias, -2.0)
        BV = {}
        for n in b_d:
            t = consts.tile([D, 1], f32, tag=f"b_{n}")
            nc.sync.dma_start(t, b_d[n][:, :])
            BV[n] = t
        for n, (kind, val) in modes.items():
            if kind == "uniform":
                t = consts.tile([D, 1], f32, tag=f"b_{n}")
                nc.vector.memset(t, val)
                BV[n] = t

        def copyback(dst, src, bname):
            """psum->sbuf copy honoring the bias mode for `bname`."""
            if modes[bname][0] == "zero":
                nc.vector.tensor_copy(dst, src)
            else:
                nc.scalar.activation(dst, src, AF.Identity, bias=BV[bname])

        def act_bias(bname):
            return 0.0 if modes[bname][0] == "zero" else BV[bname]

        ST = {}

        def phase0(b):
            """issue sample b's input DMAs one macro-step ahead."""
            st = {}
            xT = sb4.tile([128, N], bf16, tag="xT")  # [d, n]
            nc.sync.dma_start(xT[:, 0:512], xT_d[b][:, 0:512])
            nc.sync.dma_start(xT[:, 512:1024], xT_d[b][:, 512:1024])
            st["xT"] = xT
            # axT[:, 0, :] <- attnT (phase2); [:, 1, :] <- x in f8: the DR
            # moving pair for the fused u/gate matmuls.
            axT = sb4.tile([128, 2, N], f8, tag="axT")
            nc.sync.dma_start(axT[:, 1, :], x8_d[b])
            st["axT"] = axT
            return st

        def phase1a(st):
            """q/k/v projections + psum evacuation (casts at DVE queue front)."""
            xT = st["xT"]
            p_q = pw.tile([128, N], f32, tag="pw")
            nc.tensor.matmul(p_q[:, 0:512], W["Wq"], xT[:, 0:512], start=True, stop=True)
            nc.tensor.matmul(p_q[:, 512:1024], W["Wq"], xT[:, 512:1024], start=True, stop=True)
            qT = sb.tile([128, N], bf16, tag="qT")
            copyback(qT, p_q, "bq")

            p_k = pw.tile([128, N], f32, tag="pw")
            nc.tensor.matmul(p_k[:, 0:512], W["Wk"], xT[:, 0:512], start=True, stop=True)
            nc.tensor.matmul(p_k[:, 512:1024], W["Wk"], xT[:, 512:1024], start=True, stop=True)
            kT = sb.tile([128, N], bf16, tag="kT")
            copyback(kT, p_k, "bk")

            p_v = pw.tile([128, N], f32, tag="pw")
            for c in range(NT):
                nc.tensor.matmul(p_v[:, c * 128:(c + 1) * 128], xT[:, c * 128:(c + 1) * 128], W["Wv"], start=True, stop=True)
            v_nat = sb.tile([128, NT, 128], f8, tag="v_nat")
            nc.vector.tensor_copy(v_nat, p_v.rearrange("p (c n) -> p c n", c=NT))
            st["qT"], st["kT"], st["v_nat"] = qT, kT, v_nat

        def phase12(stA, stB):
            """Fused: QK^T + exp for sample A, den/recip/AV/normalize for the
            one-step-older sample B, chunk-interleaved on the PE so the DR
            blocks fill the slots where QK would otherwise wait for the
            trailing exp chain."""
            blocks = []
            if stA is not None:
                qT, kT = stA["qT"], stA["kT"]
                # exp output in fp8e4m3: bias -2 rescales exp into fp8 range;
                # the uniform e^-2 cancels between numerator and denominator.
                expw = expp.tile([128, NT, N], f8, tag="expw")  # [m', c_m, q]
                stA["expw"] = expw

                def qk_chunk(c):
                    p_l = pw.tile([128, N], f32, tag="pw")
                    kTc = kT[:, c * 128:(c + 1) * 128]
                    nc.tensor.matmul(p_l[:, 0:512], kTc, qT[:, 0:512], start=True, stop=True)
                    nc.tensor.matmul(p_l[:, 512:1024], kTc, qT[:, 512:1024], start=True, stop=True)
                    nc.scalar.activation(expw[:, c, :], p_l, AF.Exp, scale=s, bias=expbias)

            if stB is not None:
                expwB, v_nat, axT = stB["expw"], stB["v_nat"], stB["axT"]
                rb = sb.tile([128, N], f32, tag="rb")

                def den_block(h):
                    sl = slice(h * 512, (h + 1) * 512)
                    p_dn = pd.tile([128, 512], f32, tag="pden")
                    for c in range(NT // 2):
                        nc.tensor.matmul(
                            p_dn, ones_dr, expwB[:, 2 * c:2 * c + 2, sl],
                            start=(c == 0), stop=(c == NT // 2 - 1),
                            perf_mode=DR,
                        )
                    nc.vector.reciprocal_approx_fast(out=rb[:, sl], in_=p_dn)

                def av_block(h):
                    sl = slice(h * 512, (h + 1) * 512)
                    p_av = pd.tile([128, 512], f32, tag="pav")
                    for c in range(NT // 2):
                        nc.tensor.matmul(
                            p_av, v_nat[:, 2 * c:2 * c + 2, :], expwB[:, 2 * c:2 * c + 2, sl],
                            start=(c == 0), stop=(c == NT // 2 - 1),
                            perf_mode=DR,
                        )
                    nc.vector.tensor_mul(axT[:, 0, sl], p_av, rb[:, sl])

            if stA is not None and stB is not None:
                qk_chunk(0); qk_chunk(1)
                den_block(0)
                qk_chunk(2); qk_chunk(3)
                av_block(0)
                qk_chunk(4); qk_chunk(5)
                den_block(1)
                qk_chunk(6); qk_chunk(7)
                av_block(1)
            elif stA is not None:
                for c in range(NT):
                    qk_chunk(c)
            elif stB is not None:
                den_block(0); av_block(0); den_block(1); av_block(1)

        def phase3a(b, st):
            """gated tail pass A: u/gate matmuls + psum evacuation. Emitted
            FIRST in the macro step so ucopy/relu sit at the DVE queue front;
            the two g3 matmuls borrow the den/av psum banks (free until the
            fused phase12 block later in the step). (GPSIMD cannot access
            PSUM - BIR verifier - so u copy and relu stay on the DVE.)"""
            axT = st["axT"]
            u = sb.tile([128, N], f32, tag="u")
            gp = sb.tile([128, N], bf16, tag="gp")
            t = sb.tile([128, N], bf16, tag="t")
            st["u"], st["t"] = u, t
            for h in range(2):
                sl = slice(h * 512, (h + 1) * 512)
                # u = 0.5*(ret - x): one DR matmul sums Woh@attnT + Wo1mh@x8
                # (Wo, Wo1m, bo_u pre-scaled by 0.5 on the host).
                p_m = ph.tile([128, 512], f32, tag="ph")
                nc.tensor.matmul(p_m, Wu8, axT[:, :, sl], start=True, stop=True, perf_mode=DR)
                copyback(u[:, sl], p_m, "bo_u")

                p_g = ph.tile([128, 512], f32, tag="ph")
                nc.tensor.matmul(p_g, Wg8, axT[:, :, sl], start=True, stop=True, perf_mode=DR)
                if modes["bo_g"][0] == "zero":
                    nc.vector.tensor_scalar(gp[:, sl], p_g, 0.0, None, op0=OP.max)
                else:
                    nc.scalar.activation(gp[:, sl], p_g, AF.Relu, bias=act_bias("bo_g"))
            for h in range(2):
                sl = slice(h * 512, (h + 1) * 512)
                p_g3 = pd.tile([128, 512], f32, tag="pden" if h == 0 else "pav")
                nc.tensor.matmul(p_g3, W["Wg3"], gp[:, sl], start=True, stop=True)
                # gate = sigmoid(z + bg3) = 0.5 + 0.5*tanh((z + bg3)/2);
                # out = x + gate*(ret - x) = x + (1 + tanh(...))*u
                nc.scalar.activation(t[:, sl], p_g3, AF.Tanh, scale=0.5, bias=act_bias("bg3h"))

        def phase3b(b, st):
            """gated tail pass B: apply gate, residual add, store."""
            xT, u, t = st["xT"], st["u"], st["t"]
            gd = sb.tile([128, N], f32, tag="gd")
            o = sb.tile([128, N], f32, tag="o")
            for h in range(2):
                sl = slice(h * 512, (h + 1) * 512)
                nc.vector.scalar_tensor_tensor(
                    gd[:, sl], t[:, sl], 1.0, u[:, sl], op0=OP.add, op1=OP.mult,
                )
                nc.gpsimd.tensor_add(o[:, sl], gd[:, sl], xT[:, sl])
                nc.sync.dma_start(out_d[b][:, sl], o[:, sl])

        # Software pipeline, depth 4. Per macro step:
        #   P0(k):  input DMAs for sample k (a full step of prefetch),
        #   P3a(k-3): tail matmuls + evacuations (DVE queue front),
        #   P3b(k-3): gate apply + residual + store (gpsimd),
        #   P12: QK^T+exp for k-1 interleaved with den/AV for k-2 - FIRST
        #        among the heavy PE blocks so the 8-exp chain starts early
        #        (it is the longest per-step chain),
        #   P1a(k): q/k/v matmuls + casts at the step's END - their results
        #        are only needed by the next step's P12.
        for k in range(BPC + 3):
            if k < BPC:
                ST[k] = phase0(k)
            if k == 0:
                # deferred so sample 0's xT DMA beats it onto the queue;
                # wpk8 is first needed by phase3a three steps later
                nc.sync.dma_start(wpk8, wpk8_d[:, :, :])
            if 0 <= k - 3:
                phase3a(k - 3, ST[k - 3])
                phase3b(k - 3, ST[k - 3])
            phase12(
                ST[k - 1] if 0 <= k - 1 < BPC else None,
                ST[k - 2] if 0 <= k - 2 < BPC else None,
            )
            if k < BPC:
                phase1a(ST[k])

    # Force Exp and Tanh to resolve to the one table set that holds both
    # (exp_and_others): contents-only lie to the set chooser, dict order
    # (= act_func_set_id) preserved; the set actually loaded at runtime does
    # contain every ACT function used (exp, tanh, relu, identity).
    import concourse.bacc as bacc_mod

    real_get = bacc_mod.get_activation_tables
    target = "exp_and_others"

    def patched_get(arch):
        tabs = real_get(arch)
        strip = {AF.Exp, AF.Tanh}
        return {
            name: (set(fns) if name == target else set(fns) - strip)
            for name, fns in tabs.items()
        }

    bacc_mod.get_activation_tables = patched_get
    try:
        nc.compile()
    finally:
        bacc_mod.get_activation_tables = real_get
    return nc


def _prep_host(inputs):
    """Host-side folds; returns (xT bf16 [B,D,N], packed weights, biases)."""
    f32 = np.float32
    g = {k: np.asarray(v, f32) for k, v in inputs.items()}

    Woh = 0.5 * g["Wo"]
    Wo1mh = 0.5 * (g["Wo1"] - np.eye(D, dtype=f32))
    Wog2 = g["Wo"] @ g["Wg2"]                      # msg path folded into gate
    bo_msg = g["bo"] + g["bv"] @ g["Wo"]           # bv folded through Wo
    bo_u = 0.5 * (bo_msg + g["bo1"])               # 0.5*(msg bias + ret bias)
    bo_g = bo_msg @ g["Wg2"] + g["bg1"] + g["bg2"]
    bg3h = 0.5 * g["bg3"]

    wmap = {
        "Wq": g["Wq"], "Wk": g["Wk"], "Wv": g["Wv"], "Woh": Woh,
        "Wo1mh": Wo1mh, "Wg1": g["Wg1"], "Wog2": Wog2, "Wg3": g["Wg3"],
    }
    bmap = {
        "bq": g["bq"], "bk": g["bk"],
        "bo_u": bo_u, "bo_g": bo_g, "bg3h": bg3h,
    }
    bf16 = ml_dtypes.bfloat16
    f8 = ml_dtypes.float8_e4m3
    wpk = np.ascontiguousarray(
        np.stack([wmap[n] for n in WNAMES], axis=1).astype(bf16)
    )  # [D, 4, D]
    wpk8 = np.ascontiguousarray(
        np.stack([wmap[n] for n in W8NAMES], axis=1).astype(f8)
    )  # [D, 4, D]
    xt = g["x"].transpose(0, 2, 1)
    xT = np.ascontiguousarray(xt.astype(bf16))  # [B, D, N]
    x8 = np.ascontiguousarray(xt.astype(f8))    # [B, D, N]
    return xT, x8, wpk, wpk8, bmap


def _prep_inputs(inputs):
    xT, x8, wpk, wpk8, bmap = _prep_host(inputs)
    modes = {n: _bias_mode(v) for n, v in bmap.items()}
    base = {"wpk": wpk, "wpk8": wpk8}
    for n, v in bmap.items():
        if modes[n][0] == "ap":
            base[n] = np.ascontiguousarray(v.reshape(D, 1).astype(np.float32))
    in_maps = []
    for c in range(NCORES):
        m = dict(base)
        m["xT"] = np.ascontiguousarray(xT[c * BPC:(c + 1) * BPC])
        m["x8"] = np.ascontiguousarray(x8[c * BPC:(c + 1) * BPC])
        in_maps.append(m)
    return in_maps, modes


def kernel(**inputs):
    from concourse.bass_utils import run_bass_kernel_spmd

    in_maps, modes = _prep_inputs(inputs)
    key = tuple(sorted((n, k[0], k[1]) for n, k in modes.items()))
    if _CACHE.get("key") != key:
        _CACHE["nc"] = _build_nc(modes)
        _CACHE["key"] = key
    nc = _CACHE["nc"]

    res = run_bass_kernel_spmd(nc, in_maps, list(range(NCORES)))
    out = np.concatenate([r["out"] for r in res.results], axis=0)  # [B, D, N]
    return np.ascontiguousarray(out.transpose(0, 2, 1)).astype(np.float32)
